# revision 1
# baseline (speedup 1.0000x reference)
"""CenterPixelCrossAttention Trainium2 kernel.

Math: each batch item has a SINGLE query token (the center pixel), so the
attention collapses to rank-1 form:
    scores[b, t, h] = x[b, t, :] . ck[b, :, h]   with ck[b] = (Wk_h^T q_{b,h}) * sm_scale
    out_v[b, h, :]  = (sum_t softmax_t(scores) * x[b, t, :]) @ Wv_h^T
    out[b]          = concat_h(out_v) @ Wo^T + bo
The full K/V projections (64x the FLOPs) are never materialized and x is
streamed from HBM exactly once (in fp16: half the fp32 traffic; measured
output rel err ~1.5e-4 thanks to fp32 PSUM accumulation everywhere).

Distribution: data-parallel over batch, 2 batch items per NeuronCore, no
collectives.

Device pipeline, per 512-token "quad" (one 768 KB DMA: native [128,2048]
fp16 tokens-on-partitions + d-chunks 2,3 pre-transposed on the host):
  stage 1: PE transposes d-chunks 0,1 (128x128 fp16 is_transpose) -> PSUM,
           DVE copies them to SBUF
  stage 2: scores matmul (ck stationary [128d,8h], x^T moving, N=512),
           fp32 PSUM accumulate over d-chunks
  stage 3: exp on ACT (max-free softmax: logits are O(1) by construction;
           accum_out gives running row sums), PE-transpose of the exp rows
           to attn^T [t, h], one ACT copy out of PSUM
  stage 4: pass-2 matmul (attn^T stationary [128t,8h], native x moving,
           N=512) accumulating unnormalized xbar[h, d] in PSUM
The four stages are software-pipelined across quads (stage s of quad k runs
beside stage s-1 of quad k+1 ...) so the strict-FIFO engines never
head-of-line block on cross-engine handoffs.  Per batch: 1/sum
normalization folds into xbar; Wv (head-block-diagonal) and Wo projections
+ bias run once at the tail.
"""

import numpy as np
from contextlib import ExitStack

import concourse.bass as bass
import concourse.bacc as bacc
import concourse.tile as tile
from concourse import mybir
from concourse.bass_utils import run_bass_kernel_spmd

F32 = mybir.dt.float32
F16 = mybir.dt.float16

B, N, DIM, HEADS, DHEAD = 16, 4096, 512, 8, 64
NCORES = 8
BPC = B // NCORES          # batch items per core = 2
NQ = 8                     # quads (512-token groups) per batch item
QT = 512                   # tokens per quad
NT = 4                     # 128-token sub-tiles per quad
NJ = 4                     # 128-wide d-chunks
NPT = 2                    # d-chunks transposed on the PE (rest on the host)
XW = QT * (NT + NJ - NPT)  # x row width: native 2048 + pre-transposed 1024

TRACE = False              # test.py flips this for profiling runs
LAST_RESULTS = None        # stash of BassKernelResults for test.py


def build_program(reps=1):
    DT = F16
    nc = bacc.Bacc("TRN2", target_bir_lowering=False, debug=False,
                   num_devices=NCORES)

    x_d = nc.dram_tensor("x", [BPC, NQ, 128, XW], DT, kind="ExternalInput")
    ck_d = nc.dram_tensor("ck", [128, BPC * NJ * HEADS], DT, kind="ExternalInput")
    wvt_d = nc.dram_tensor("wvt", [NJ, 128, DIM], DT, kind="ExternalInput")
    wot_d = nc.dram_tensor("wot", [NJ, 128, DIM], DT, kind="ExternalInput")
    bo_d = nc.dram_tensor("bo", [128, NJ], F32, kind="ExternalInput")
    id_d = nc.dram_tensor("ident", [128, 128], DT, kind="ExternalInput")
    out_d = nc.dram_tensor("out", [128, NJ * BPC], F32, kind="ExternalOutput")

    with tile.TileContext(nc) as tc, ExitStack() as ctx:
        const = ctx.enter_context(tc.tile_pool(name="const", bufs=1))
        xq_pool = ctx.enter_context(tc.tile_pool(name="xq", bufs=10))
        xt_pool = ctx.enter_context(tc.tile_pool(name="xt", bufs=8))
        e_pool = ctx.enter_context(tc.tile_pool(name="e", bufs=4))
        at_pool = ctx.enter_context(tc.tile_pool(name="at", bufs=6))
        ps_xt = ctx.enter_context(tc.tile_pool(name="ps_xt", bufs=3, space="PSUM"))
        ps_sc = ctx.enter_context(tc.tile_pool(name="ps_sc", bufs=3, space="PSUM"))
        ps_at = ps_xt
        ps_xb = ctx.enter_context(tc.tile_pool(name="ps_xb", bufs=2, space="PSUM"))

        ident = const.tile([128, 128], DT)
        nc.scalar.dma_start(ident[:], id_d.ap()[:, :])
        ck_sb = const.tile([128, BPC * NJ * HEADS], DT)
        nc.scalar.dma_start(ck_sb[:], ck_d.ap()[:, :])
        wvt_sb = const.tile([128, NJ * DIM], DT)
        wot_sb = const.tile([128, NJ * DIM], DT)
        bo_sb = const.tile([128, NJ], F32)

        sums_sb = const.tile([8, BPC * NQ], F32)
        sinv_sb = const.tile([8, BPC], F32)
        junk_sb = const.tile([8, NQ], F32)
        xbar_sb = const.tile([8, BPC * DIM], DT)
        xbarT = const.tile([128, NJ * BPC * HEADS], DT)
        v_all = const.tile([128, NJ * BPC], DT)
        o_sb = const.tile([128, NJ * BPC], F32)

        for _rep in range(reps):
            ps_xbar = [ps_xb.tile([8, DIM], F32, tag="xbar", name=f"xbar{i}")
                       for i in range(BPC)]

            def scores_phase(b, q, ps_s, xts):
                # scores accumulate over d-chunks: [8, 512]
                for j in range(NJ):
                    nc.tensor.matmul(
                        ps_s[:],
                        ck_sb[:, (b * NJ + j) * HEADS:(b * NJ + j + 1) * HEADS],
                        xts[j][:],
                        start=(j == 0),
                        stop=(j == NJ - 1),
                    )

            def attn_a(b, q, ps_s):
                # exp (+ running row-sums), transpose attn to [t, h]
                e_sb = e_pool.tile([8, QT], DT, tag="e", name="e_sb")
                nc.scalar.activation(
                    e_sb[:], ps_s[:], mybir.ActivationFunctionType.Exp,
                    accum_out=sums_sb[:, b * NQ + q: b * NQ + q + 1],
                )
                pat = ps_at.tile([128, NT * 8], DT, tag="pxt", name="pat")
                for s in range(NT):
                    nc.tensor.matmul(
                        pat[:, s * 8:(s + 1) * 8],
                        e_sb[:, s * 128:(s + 1) * 128],
                        ident[0:8, 0:8],
                        is_transpose=True,
                    )
                at_sb = at_pool.tile([128, NT * 8], DT, tag="at", name="at_sb")
                nc.vector.tensor_copy(at_sb[:], pat[:])
                return at_sb

            def attn_b(b, q, at_sb, xq):
                for s in range(NT):
                    # xbar[h, d] += attn^T.T @ x  (contraction over tokens)
                    nc.tensor.matmul(
                        ps_xbar[b][:],
                        at_sb[:, s * 8:(s + 1) * 8],
                        xq[:, s * QT:(s + 1) * QT],
                        start=(q == 0 and s == 0),
                        stop=(q == NQ - 1 and s == NT - 1),
                    )

            def batch_tail(b):
                # sums -> 1/sum, normalize xbar, transpose to [d, (b,h)]
                nc.vector.tensor_scalar(
                    junk_sb[:], sums_sb[:, b * NQ:(b + 1) * NQ], 0.0, None,
                    mybir.AluOpType.add, mybir.AluOpType.add,
                    accum_out=sinv_sb[:, b:b + 1],
                )
                nc.vector.reciprocal(sinv_sb[:, b:b + 1], sinv_sb[:, b:b + 1])
                nc.vector.tensor_scalar_mul(
                    xbar_sb[:, b * DIM:(b + 1) * DIM], ps_xbar[b][:],
                    sinv_sb[:, b:b + 1],
                )
                for j in range(NJ):
                    pt = ps_at.tile([128, NT * 8], DT, tag="pxt", name="pt")
                    nc.tensor.matmul(
                        pt[:, 0:8],
                        xbar_sb[:, b * DIM + j * 128: b * DIM + (j + 1) * 128],
                        ident[0:8, 0:8],
                        is_transpose=True,
                    )
                    nc.scalar.copy(
                        xbarT[:, j * BPC * 8 + b * 8: j * BPC * 8 + (b + 1) * 8],
                        pt[:, 0:8])

            # 4-stage software pipeline over quads (see module docstring)
            S1 = S2 = S3 = None
            quads = [(b, q) for b in range(BPC) for q in range(NQ)]
            for item in quads + [None] * 3:
                if item is not None:
                    b, q = item
                    xq = xq_pool.tile([128, XW], DT, tag="xq")
                    if q == 0 and b == 0:
                        # split the very first load so the PE starts sooner
                        W = XW // 4
                        for s in range(4):
                            nc.sync.dma_start(
                                xq[:, s * W:(s + 1) * W],
                                x_d.ap()[b, q][:, s * W:(s + 1) * W])
                    else:
                        nc.sync.dma_start(xq[:], x_d.ap()[b, q])

                    ps_s = ps_sc.tile([8, QT], F32, tag="sc", name="ps_s")
                    xts = []
                    for j in range(NJ):
                        if j >= NPT:
                            # chunks >= NPT sit pre-transposed in the load tail
                            k = NT + (j - NPT)
                            xts.append(xq[:, k * QT:(k + 1) * QT])
                            continue
                        xt = xt_pool.tile([128, QT], DT, tag="xt", name="xt")
                        pxt = ps_xt.tile([128, QT], DT, tag="pxt", name="pxt")
                        for s in range(NT):
                            # psum slot s = (x chunk [t,d]).T
                            nc.tensor.matmul(
                                pxt[:, s * 128:(s + 1) * 128],
                                xq[:, s * QT + j * 128: s * QT + (j + 1) * 128],
                                ident[:],
                                is_transpose=True,
                            )
                        nc.vector.tensor_copy(xt[:], pxt[:])
                        xts.append(xt)
                    new_S1 = (b, q, ps_s, xts, xq)
                else:
                    new_S1 = None

                if S1 is not None:
                    scores_phase(*S1[:4])
                new_S3 = None
                if S2 is not None:
                    at_sb = attn_a(S2[0], S2[1], S2[2])
                    new_S3 = (S2[0], S2[1], at_sb, S2[3])
                if S3 is not None:
                    attn_b(*S3)
                    if S3[1] == NQ - 1:
                        batch_tail(S3[0])
                S3 = new_S3
                S2 = (S1[0], S1[1], S1[2], S1[4]) if S1 is not None else None
                S1 = new_S1

            for j in range(NJ):
                nc.scalar.dma_start(wvt_sb[:, j * DIM:(j + 1) * DIM], wvt_d.ap()[j])
                nc.scalar.dma_start(wot_sb[:, j * DIM:(j + 1) * DIM], wot_d.ap()[j])
            nc.scalar.dma_start(bo_sb[:], bo_d.ap()[:, :])

            # v projection: v[i, (b,h)] = sum_d WvT[d, i] * xbarT[d, (b,h)]
            for ji in range(NJ):
                pv = ps_xt.tile([128, BPC * HEADS], F32, tag="pxt", name="pv")
                for jd in range(NJ):
                    nc.tensor.matmul(
                        pv[:],
                        wvt_sb[:, jd * DIM + ji * 128: jd * DIM + (ji + 1) * 128],
                        xbarT[:, jd * BPC * 8:(jd + 1) * BPC * 8],
                        start=(jd == 0),
                        stop=(jd == NJ - 1),
                    )
                # head-diagonal extraction: i-chunk ji covers heads 2ji
                # (rows 0-63) and 2ji+1 (rows 64-127); batch b at column b*8+h
                for bb in range(BPC):
                    nc.vector.tensor_copy(
                        v_all[0:64, ji * BPC + bb: ji * BPC + bb + 1],
                        pv[0:64, bb * 8 + 2 * ji: bb * 8 + 2 * ji + 1],
                    )
                    nc.vector.tensor_copy(
                        v_all[64:128, ji * BPC + bb: ji * BPC + bb + 1],
                        pv[64:128, bb * 8 + 2 * ji + 1: bb * 8 + 2 * ji + 2],
                    )

            # out projection: out[dim, b] = sum_i WoT[i, dim] * v[i, b]
            for jd in range(NJ):
                po = ps_sc.tile([128, BPC], F32, tag="sc", name="po")
                for ji in range(NJ):
                    nc.tensor.matmul(
                        po[:],
                        wot_sb[:, ji * DIM + jd * 128: ji * DIM + (jd + 1) * 128],
                        v_all[:, ji * BPC:(ji + 1) * BPC],
                        start=(ji == 0),
                        stop=(ji == NJ - 1),
                    )
                nc.scalar.activation(
                    o_sb[:, jd * BPC:(jd + 1) * BPC], po[:],
                    mybir.ActivationFunctionType.Identity,
                    bias=bo_sb[:, jd:jd + 1],
                )
            nc.sync.dma_start(out_d.ap()[:, :], o_sb[:])

    nc.compile()
    return nc


def kernel(**inputs):
    global LAST_RESULTS
    x = np.ascontiguousarray(np.asarray(inputs["x"], dtype=np.float32))
    Wq = np.asarray(inputs["Wq"], dtype=np.float32)
    Wk = np.asarray(inputs["Wk"], dtype=np.float32)
    Wv = np.asarray(inputs["Wv"], dtype=np.float32)
    Wo = np.asarray(inputs["Wo"], dtype=np.float32)
    bo = np.asarray(inputs["bo"], dtype=np.float32)
    pi = np.asarray(inputs["patch_indices"]).astype(np.int64)
    scale = np.asarray(inputs["scale"]).astype(np.int64)

    idx = pi[:, 0] * scale[1] + pi[:, 1]
    sel = x[np.arange(B), idx]                       # [B, DIM]
    q = (sel @ Wq.T).reshape(B, HEADS, DHEAD)        # [B, h, dh]
    # ck[b, d, h] = sum_i q[b,h,i] * Wk[h*64+i, d], scaled by 1/sqrt(dh)
    ck = np.einsum("bhi,hid->bdh", q, Wk.reshape(HEADS, DHEAD, DIM),
                   dtype=np.float32).astype(np.float32) * np.float32(DHEAD ** -0.5)

    wvt = np.ascontiguousarray(Wv.T.reshape(NJ, 128, DIM)).astype(np.float16)
    wot = np.ascontiguousarray(Wo.T.reshape(NJ, 128, DIM)).astype(np.float16)
    bo_r = np.ascontiguousarray(bo.reshape(NJ, 128).T)
    ident = np.eye(128, dtype=np.float16)

    x16 = x.astype(np.float16)
    in_maps = []
    for c in range(NCORES):
        xsf = x16[c * BPC:(c + 1) * BPC]             # [2, 4096, 512] fp16
        xs_nat = xsf.reshape(BPC, NQ, NT, 128, DIM).transpose(0, 1, 3, 2, 4)
        xs_nat = xs_nat.reshape(BPC, NQ, 128, NT * DIM)
        # d-chunks >= NPT pre-transposed to [d, t] on the host
        xtt = xsf.reshape(BPC, NQ, QT, DIM)[:, :, :, NPT * 128:]
        xtt = xtt.transpose(0, 1, 3, 2)              # [2, 8, (NJ-NPT)*128, 512]
        xtt = xtt.reshape(BPC, NQ, NJ - NPT, 128, QT).transpose(0, 1, 3, 2, 4)
        xtt = xtt.reshape(BPC, NQ, 128, (NJ - NPT) * QT)
        xs = np.ascontiguousarray(np.concatenate([xs_nat, xtt], axis=3))
        ckc = np.empty((128, BPC * NJ * HEADS), dtype=np.float16)
        for bb in range(BPC):
            for j in range(NJ):
                ckc[:, (bb * NJ + j) * HEADS:(bb * NJ + j + 1) * HEADS] = \
                    ck[c * BPC + bb, j * 128:(j + 1) * 128, :]
        in_maps.append({
            "x": xs, "ck": ckc, "wvt": wvt, "wot": wot,
            "bo": bo_r, "ident": ident,
        })

    nc = build_program()
    res = run_bass_kernel_spmd(nc, in_maps, list(range(NCORES)), trace=TRACE)
    LAST_RESULTS = res

    out = np.empty((B, 1, DIM), dtype=np.float32)
    for c in range(NCORES):
        oc = res.results[c]["out"]                   # [128, NJ*BPC]
        for bb in range(BPC):
            out[c * BPC + bb, 0, :] = oc[:, bb::BPC].T.reshape(DIM)
    return out



# revision 8
# speedup vs baseline: 2.1890x; 2.1890x over previous
"""CenterPixelCrossAttention Trainium2 kernel (v2: single fp8 x stream).

Math: one query token per batch item makes the attention rank-1:
    scoresT[t, h] = x[t, :] . ck[:, h]      ck[b] = (Wk_h^T q_{b,h}) * sm_scale
    xbarT[d, h]   = sum_t exp(scoresT)[t, h] * x[t, d]   (unnormalized)
    out[b]        = ((xbarT / sums) per-head @ Wv_h^T) @ Wo^T + bo
The full K/V projections are never materialized.  x is streamed from HBM
exactly ONCE, in fp8-e4m3 (1/4 of the baseline fp32 bytes); all matmuls
accumulate in fp32 PSUM, so the measured output rel err stays ~5e-3
(gate: 2e-2).  The tiny tail projections (Wv/Wo/bo, 0.5 MFLOP) and the
1/sum normalization run on the host in fp32 - shipping Wv/Wo to every
core would cost more DMA time than the whole rest of the kernel tail.

Layout trick (pair-transpose + d-combs): pass 1 needs x with d on
partitions, pass 2 needs tokens on partitions.  Instead of shipping both
layouts (2x DMA) we ship native [t, d] fp8 once and transpose on the PE
viewing the fp8 tile as fp16 PAIRS: transposing [128 t, 128 pairs] moves
2 bytes/lane/cycle (half the instructions of element-wise fp8
transposes).  A pair-transposed block holds d = base + 2u + c at
partition u, free position 2t + c - so for fixed parity c the partition
axis is a stride-2 "comb" of d values.  Packing ck rows into the same
combs on the host makes the scores matmul contraction line up exactly;
scoresT comes out in true [token, head] order, which is exactly the
moving operand pass 2 wants.  Per 512-token quad the PE does only
8 x 128-col pair-transposes + 16 x 8-col score matmuls + 16 x 8-col
pass-2 matmuls + 4 x 1-col sum matmuls: ~1.2k cycles, just under the
728 ns/quad DMA pace.

Distribution: data-parallel over batch, 2 batch items per core, no
collectives.  Output (xbarT + row sums, [128, 33] fp32 per batch item)
is DMA'd straight from PSUM.
"""

import numpy as np
import ml_dtypes
from contextlib import ExitStack

import concourse.bass as bass
import concourse.bacc as bacc
import concourse.tile as tile
from concourse import mybir
from concourse.bass_utils import run_bass_kernel_spmd

F32 = mybir.dt.float32
F16 = mybir.dt.float16
F8 = mybir.dt.float8e4
E4 = ml_dtypes.float8_e4m3           # numpy dtype matching mybir float8e4

B, N, DIM, HEADS, DHEAD = 16, 4096, 512, 8, 64
NCORES = 8
BPC = B // NCORES          # batch items per core = 2
NQ = 8                     # 512-token quads per batch item
QT = 512                   # tokens per quad
NT = 4                     # 128-token sub-tiles per quad
NJ = 4                     # 128-wide d-chunks
NQUADS = BPC * NQ          # 16 quad tiles per core
QW = NT * DIM              # 2048 fp8 cols per quad tile

# const region: ident fp16 (256 B) | ck combs (BPC*4*8 B) | ones (1 B)
CK_OFF = 256
ONES_OFF = CK_OFF + BPC * 4 * HEADS
CW = ONES_OFF + 2          # 322 (padded even so fp16 bitcasts stay aligned)
XCOLS = CW + NQUADS * QW

# DMA load plan: quads per dma_start.  Small first groups shorten the
# pipeline fill; small last groups shorten the drain chain.
LOAD_PLAN = [1, 1, 2, 2, 2, 2, 2, 2, 1, 1]

TRACE = False              # test.py flips this for profiling runs
LAST_RESULTS = None        # stash of BassKernelResults for test.py


def build_program(reps=1):
    nc = bacc.Bacc("TRN2", target_bir_lowering=False, debug=False,
                   num_devices=NCORES)

    x_d = nc.dram_tensor("x", [128, XCOLS], F8, kind="ExternalInput")
    out_d = nc.dram_tensor("out", [128, BPC * 33], F32, kind="ExternalOutput")

    quad_col = [CW + k * QW for k in range(NQUADS)]
    load_start = np.cumsum([0] + LOAD_PLAN)[:-1]        # first quad of group

    with tile.TileContext(nc) as tc, ExitStack() as ctx:
        const = ctx.enter_context(tc.tile_pool(name="const", bufs=1))
        xq_pool = ctx.enter_context(tc.tile_pool(name="xq", bufs=4))
        xq2_pool = ctx.enter_context(tc.tile_pool(name="xq2", bufs=3))
        xt_pool = ctx.enter_context(tc.tile_pool(name="xt", bufs=3))
        e_pool = ctx.enter_context(tc.tile_pool(name="e", bufs=4))
        ps_xt = ctx.enter_context(tc.tile_pool(name="ps_xt", bufs=3, space="PSUM"))
        ps_sc = ctx.enter_context(tc.tile_pool(name="ps_sc", bufs=3, space="PSUM"))
        ps_xb = ctx.enter_context(tc.tile_pool(name="ps_xb", bufs=2, space="PSUM"))

        const_sb = const.tile([128, CW], F8)
        o_sb = const.tile([128, BPC * 33], F32)
        ident16 = const_sb[:, 0:CK_OFF].bitcast(F16)     # [128, 128]
        ones8 = const_sb[:, ONES_OFF:ONES_OFF + 1]       # [128, 1]

        def ck8(b, m):
            o = CK_OFF + (b * 4 + m) * HEADS
            return const_sb[:, o:o + HEADS]              # [128, 8]

        for _rep in range(reps):
            ps_xbar = [ps_xb.tile([128, 33], F32, tag="xbar", name=f"xbar{i}")
                       for i in range(BPC)]

            # quad k state (filled by the pipeline stages below)
            xq_slices = [None] * NQUADS   # native fp8 [128, 2048] view
            xt_tiles = [None] * NQUADS    # pair-transposed fp16 tile
            sc_tiles = [None] * NQUADS    # scoresT psum [128, 32] f32
            e_tiles = [None] * NQUADS     # exp(scoresT) fp8 [128, 32]
            gi = 0

            def stage_load(i):
                nonlocal gi
                if gi < len(LOAD_PLAN) and load_start[gi] == i:
                    nq = LOAD_PLAN[gi]
                    c0 = quad_col[i] if gi > 0 else 0     # group 0 carries const
                    c1 = quad_col[i] + nq * QW
                    if gi == 0:
                        t = xq_pool.tile([128, CW + QW], F8, tag="xq1c", name="xqc")
                        nc.sync.dma_start(t[:], x_d.ap()[:, c0:c1])
                        # const region is a slice of the same tile; record a
                        # full-tile view so slices below alias it
                        nc.vector.tensor_copy(const_sb[:], t[:, 0:CW])
                        xq_slices[i] = t[:, CW:CW + QW]
                    elif nq == 1:
                        t = xq_pool.tile([128, QW], F8, tag="xq1", name="xq1")
                        nc.sync.dma_start(t[:], x_d.ap()[:, c0:c1])
                        xq_slices[i] = t[:]
                    else:
                        t = xq2_pool.tile([128, nq * QW], F8, tag="xq2", name="xq2")
                        nc.sync.dma_start(t[:], x_d.ap()[:, c0:c1])
                        for k in range(nq):
                            xq_slices[i + k] = t[:, k * QW:(k + 1) * QW]
                    gi += 1

            def stage_transpose(i):
                # 8 fp16-pair transposes: [128 t, 128 pairs] -> PSUM
                xq16 = xq_slices[i].bitcast(F16)          # [128, 1024]
                pxt = ps_xt.tile([128, 1024], F16, tag="pxt", name="pxt")
                for blk in range(8):
                    nc.tensor.matmul(
                        pxt[:, blk * 128:(blk + 1) * 128],
                        xq16[:, blk * 128:(blk + 1) * 128],
                        ident16,
                        is_transpose=True,
                    )
                xt = xt_pool.tile([128, 1024], F16, tag="xt", name="xt")
                nc.vector.tensor_copy(xt[:], pxt[:])
                xt_tiles[i] = xt

            def stage_scores(i):
                b = i // NQ
                xt8 = xt_tiles[i][:].bitcast(F8)          # [128, 2048]
                sc = ps_sc.tile([128, 32], F32, tag="sc", name="sc")
                # one start/stop per PSUM zero region: the start marks the
                # whole 2KB bank pending-zero, so every slice's first touch
                # self-initializes; extra starts would wipe sibling slices.
                for s in range(NT):
                    for m in range(4):                    # comb (g, c): m = g*2+c
                        g, c = m >> 1, m & 1
                        blk = s * 2 + g
                        nc.tensor.matmul(
                            sc[:, s * 8:(s + 1) * 8],
                            xt8[:, blk * 256 + c: blk * 256 + 256: 2],
                            ck8(b, m),
                            start=(s == 0 and m == 0),
                            stop=(s == NT - 1 and m == 3),
                        )
                sc_tiles[i] = sc
                e8 = e_pool.tile([128, 32], F8, tag="e", name="e8")
                nc.scalar.activation(e8[:], sc[:],
                                     mybir.ActivationFunctionType.Exp)
                e_tiles[i] = e8

            def stage_accum(i):
                b, q = i // NQ, i % NQ
                e8 = e_tiles[i]
                # single start (very first matmul, q==0) / single stop (very
                # last, q==NQ-1) for the whole xbar+sums bank - see above.
                for s in range(NT):
                    nc.tensor.matmul(
                        ps_xbar[b][0:8, 32:33],
                        e8[:, s * 8:(s + 1) * 8],
                        ones8,
                        start=(q == 0 and s == 0),
                        stop=False,
                    )
                for s in range(NT):
                    for j in range(NJ):
                        nc.tensor.matmul(
                            ps_xbar[b][:, j * 8:(j + 1) * 8],
                            xq_slices[i][:, s * DIM + j * 128: s * DIM + (j + 1) * 128],
                            e8[:, s * 8:(s + 1) * 8],
                            start=False,
                            stop=(q == NQ - 1 and s == NT - 1 and j == NJ - 1),
                        )
                if q == NQ - 1:
                    # batch done: ship xbarT + sums
                    nc.vector.tensor_copy(o_sb[:, b * 33:(b + 1) * 33],
                                          ps_xbar[b][:])
                    nc.sync.dma_start(out_d.ap()[:, b * 33:(b + 1) * 33],
                                      o_sb[:, b * 33:(b + 1) * 33])

            for i in range(NQUADS + 2):
                if i < NQUADS:
                    stage_load(i)
                    stage_transpose(i)
                if 1 <= i <= NQUADS:
                    stage_scores(i - 1)
                if i >= 2:
                    stage_accum(i - 2)

    nc.compile()
    return nc


def kernel(**inputs):
    global LAST_RESULTS
    x = np.ascontiguousarray(np.asarray(inputs["x"], dtype=np.float32))
    Wq = np.asarray(inputs["Wq"], dtype=np.float32)
    Wk = np.asarray(inputs["Wk"], dtype=np.float32)
    Wv = np.asarray(inputs["Wv"], dtype=np.float32)
    Wo = np.asarray(inputs["Wo"], dtype=np.float32)
    bo = np.asarray(inputs["bo"], dtype=np.float32)
    pi = np.asarray(inputs["patch_indices"]).astype(np.int64)
    scale = np.asarray(inputs["scale"]).astype(np.int64)

    idx = pi[:, 0] * scale[1] + pi[:, 1]
    sel = x[np.arange(B), idx]                       # [B, DIM]
    q = (sel @ Wq.T).reshape(B, HEADS, DHEAD)        # [B, h, dh]
    # ck[b, d, h] = sum_i q[b,h,i] * Wk[h*64+i, d], scaled by 1/sqrt(dh)
    ck = np.einsum("bhi,hid->bdh", q, Wk.reshape(HEADS, DHEAD, DIM),
                   dtype=np.float32) * np.float32(DHEAD ** -0.5)
    ck8 = ck.astype(E4)                              # [B, 512, 8]
    # comb packing: partition u of comb m=(g,c) holds d = g*256 + 2u + c
    ckc = ck8.reshape(B, 2, 128, 2, HEADS).transpose(0, 2, 1, 3, 4)
    ckc = np.ascontiguousarray(ckc.reshape(B, 128, 4 * HEADS))

    x8 = x.astype(E4)                                # [B, 4096, 512]
    # native quad layout [b, q, p, s*512 + d]
    x_nat = x8.reshape(B, NQ, NT, 128, DIM).transpose(0, 1, 3, 2, 4)
    x_nat = x_nat.reshape(B, NQ, 128, QW)

    ident = np.eye(128, dtype=np.float16)
    ident8 = np.ascontiguousarray(ident).view(E4)    # [128, 256] raw bytes
    ones8 = np.ones((128, 1), dtype=E4)

    in_maps = []
    for c in range(NCORES):
        xall = np.empty((128, XCOLS), dtype=E4)
        xall[:, 0:CK_OFF] = ident8
        for bb in range(BPC):
            xall[:, CK_OFF + bb * 4 * HEADS: CK_OFF + (bb + 1) * 4 * HEADS] = \
                ckc[c * BPC + bb]
        xall[:, ONES_OFF:ONES_OFF + 1] = ones8
        xall[:, ONES_OFF + 1:CW] = np.zeros((128, 1), dtype=E4)
        for k in range(NQUADS):
            bb, qq = k // NQ, k % NQ
            xall[:, CW + k * QW: CW + (k + 1) * QW] = x_nat[c * BPC + bb, qq]
        in_maps.append({"x": xall})

    nc = build_program()
    res = run_bass_kernel_spmd(nc, in_maps, list(range(NCORES)), trace=TRACE)
    LAST_RESULTS = res

    # host tail: normalize, per-head Wv, then Wo + bias (all fp32, exact)
    Wv_h = Wv.reshape(HEADS, DHEAD, DIM)             # [h, i, d]
    out = np.empty((B, 1, DIM), dtype=np.float32)
    for c in range(NCORES):
        oc = np.asarray(res.results[c]["out"], dtype=np.float32)  # [128, 66]
        for bb in range(BPC):
            blk = oc[:, bb * 33:(bb + 1) * 33]
            sums = blk[0:8, 32]                      # [h]
            xbarT = blk[:, 0:32].reshape(128, NJ, HEADS)
            # xbar[h, d = j*128 + p] = xbarT[p, j, h] / sums[h]
            xbar = xbarT.transpose(2, 1, 0).reshape(HEADS, DIM) / sums[:, None]
            v = np.einsum("hd,hid->hi", xbar, Wv_h)  # [h, i]
            out[c * BPC + bb, 0, :] = v.reshape(DIM) @ Wo.T + bo
    return out
